# revision 32
# baseline (speedup 1.0000x reference)
"""Trainium2 Bass kernel for nn_DiaMultiDense.

Computes, for s:[B,1024] f32, gold:[B,20] int, pos:[B] int and MLP weights:
    h  = relu(s @ W1 + b1)
    h  = leaky_relu(h @ W2 + b2, 0.2)
    logits[b,a,w] = h @ Wl[a,:,w] + bl[a,w]          (A=512, w in {0,1})
    pred[b,a] = argmax_w logits[b,a,w]               (-> float 0/1)
    proc[b,a] = 1 if a in gold[b, :pos[b]] else 0
    tgt = one_hot pairs of proc; loss = -mean(tgt*logsig(x) + (1-tgt)*logsig(-x))
Returns (loss, pred).

Strategy: pure data parallel over 8 NeuronCores (2048 rows each).  All
three matmul stages run as 3-pass fp16 hi/lo decompositions on the PE
(1 cyc/row vs 4 for fp32) with power-of-two scaling (x2048 per operand,
products at 2^22) that keeps every fp16 operand in normal range; the
hi/lo splits of the intermediate activations are produced on ACT/DVE
off the critical path.  This reproduces fp32-level logits (~1 argmax
flip in 8.4M vs an exact-fp32 oracle).  Loss terms are accumulated
on-chip (softplus via Exp+Ln LUTs, row-sums via ACT accumulators); the
per-row target union mask is built with a gpsimd local_scatter of ones
at host-deduplicated indices, with masked/duplicate slots redirected to
pad columns 512..531.  Host work is only sharding, dtype/layout
marshalling of inputs, and the final 8-way scalar reduction.
"""

import sys

sys.path.insert(0, "/opt/trn_rl_repo")

import numpy as np

import concourse.bacc as bacc
import concourse.mybir as mybir
import concourse.tile as tile
from concourse.bass_utils import run_bass_kernel_spmd

AF = mybir.ActivationFunctionType

# Pin every ACT function we use to the one LUT set that contains them all
# (natural_log_exp_and_others).  The default first-match table assignment
# alternates between act-func sets (Ln -> natural_log, Exp -> ..._exp_...),
# inserting ~40 mid-kernel ACT_TABLE_LOADs (~51us).  Removing our funcs
# from every other set makes the keep-set the unique match; set contents
# are only used for placement, the runtime table bytes come from
# act_info.json by index, so this is purely a scheduling hint.
_KEEP_TABLE = "natural_log_exp_and_others"
_PINNED = {AF.Relu, AF.Prelu, AF.Identity, AF.Exp, AF.Ln, AF.Copy}


def _pinned_tables(arch):
    from concourse.hw_specs import get_activation_tables
    orig = get_activation_tables(arch)
    return {name: (set(funcs) if name == _KEEP_TABLE else set(funcs) - _PINNED)
            for name, funcs in orig.items()}


bacc.get_activation_tables = _pinned_tables

ALU = mybir.AluOpType
F32 = mybir.dt.float32
F16 = mybir.dt.float16
BF16 = mybir.dt.bfloat16
I16 = mybir.dt.int16

B_FULL = 16384
S_DIM = 1024
H_DIM = 1024
H4 = 128
A_DIM = 512
MAX_LEN = 20
N_CORES = 8
TB = 512                      # batch rows per pipeline tile
NE = 544                      # scatter row width: 512 actions + 32 pad slots
SC = np.float32(2048.0)       # 2**11 operand scaling for the fp16 splits
DS11 = float(2.0 ** -11)      # psum(2^22) -> hi-split(2^11) descale
DS22 = float(2.0 ** -22)      # psum(2^22) -> true value descale

_cache = {}


def _build(Bc, with_bl, with_b1):
    nt = Bc // TB
    KT = S_DIM // 128          # contraction tiles for stages 1/2
    G = Bc // 128              # 128-row output chunks per core
    MT = H_DIM // 128

    nc = bacc.Bacc(None, target_bir_lowering=False)

    sh_d = nc.dram_tensor("sh", [S_DIM, Bc], F16, kind="ExternalInput")
    sl_d = nc.dram_tensor("sl", [S_DIM, Bc], F16, kind="ExternalInput")
    w1h_d = nc.dram_tensor("w1h", [S_DIM, H_DIM], F16, kind="ExternalInput")
    w1l_d = nc.dram_tensor("w1l", [S_DIM, H_DIM], F16, kind="ExternalInput")
    w2h_d = nc.dram_tensor("w2h", [H_DIM, H4], F16, kind="ExternalInput")
    w2l_d = nc.dram_tensor("w2l", [H_DIM, H4], F16, kind="ExternalInput")
    # [We_hi, Wo_hi] and [We_lo, Wo_lo] stacked on the free dim
    w3h_d = nc.dram_tensor("w3h", [H4, 2 * A_DIM], F16, kind="ExternalInput")
    w3l_d = nc.dram_tensor("w3l", [H4, 2 * A_DIM], F16, kind="ExternalInput")
    b1_d = nc.dram_tensor("b1", [128, 2 * MT], F32, kind="ExternalInput")  # [b1, b1*2048]
    b2_d = nc.dram_tensor("b2", [128, 2], F32, kind="ExternalInput")       # [b2, b2*2048]
    idx_d = nc.dram_tensor("idx", [Bc, MAX_LEN], I16, kind="ExternalInput")
    if with_bl:
        bl_d = nc.dram_tensor("bl", [1, 2 * A_DIM], F32, kind="ExternalInput")  # *2^22
    pred_d = nc.dram_tensor("pred", [Bc, A_DIM], F32, kind="ExternalOutput")
    acc_d = nc.dram_tensor("acc", [3, 128, G], F32, kind="ExternalOutput")

    with tile.TileContext(nc) as tc:
        with (
            tc.tile_pool(name="wpool", bufs=1) as wpool,
            tc.tile_pool(name="spool", bufs=3) as spool,
            tc.tile_pool(name="hpool", bufs=1) as hpool,
            tc.tile_pool(name="h2pool", bufs=1) as h2pool,
            tc.tile_pool(name="cpool", bufs=3) as cpool,
            tc.tile_pool(name="psA", bufs=3, space="PSUM") as psA,
            tc.tile_pool(name="psB", bufs=1, space="PSUM") as psB,
            tc.tile_pool(name="psC", bufs=2, space="PSUM") as psC,
        ):
            # ---- persistent weights / constants -------------------------
            # stage-1 weights on two queues so the sync queue only carries
            # the streamed activations
            w1hs = wpool.tile([128, KT, H_DIM], F16, tag="w1hs")
            w1ls = wpool.tile([128, KT, H_DIM], F16, tag="w1ls")
            for kt in range(KT):
                nc.scalar.dma_start(w1hs[:, kt, :], w1h_d[kt * 128:(kt + 1) * 128, :])
                nc.gpsimd.dma_start(w1ls[:, kt, :], w1l_d[kt * 128:(kt + 1) * 128, :])
            b1s = wpool.tile([128, 2 * MT], F32, tag="b1s")
            nc.scalar.dma_start(b1s[:], b1_d[:])
            b2s = wpool.tile([128, 2], F32, tag="b2s")
            nc.scalar.dma_start(b2s[:], b2_d[:])
            w2hs = wpool.tile([128, KT, H4], F16, tag="w2hs")
            w2ls = wpool.tile([128, KT, H4], F16, tag="w2ls")
            w3hs = wpool.tile([128, 2 * A_DIM], F16, tag="w3hs")
            w3ls = wpool.tile([128, 2 * A_DIM], F16, tag="w3ls")
            idxs = wpool.tile([128, G, MAX_LEN], I16, tag="idxs")

            def load_late_weights():
                # stage-2/3 weights + indices (~1.2MB) are first used ~60us
                # in; deferring their DMA emission keeps the HBM-bound tile-0
                # ramp dedicated to stage-1 weights and activations.
                nc.scalar.dma_start(w2hs[:], w2h_d.rearrange("(t p) m -> p t m", p=128))
                nc.scalar.dma_start(w2ls[:], w2l_d.rearrange("(t p) m -> p t m", p=128))
                nc.scalar.dma_start(w3hs[:], w3h_d[:])
                nc.scalar.dma_start(w3ls[:], w3l_d[:])
                nc.scalar.dma_start(idxs[:], idx_d.rearrange("(g p) l -> p g l", p=128))
            ones20 = wpool.tile([128, MAX_LEN], BF16, tag="ones20")
            nc.vector.memset(ones20[:], 1.0)
            if with_bl:
                ones1 = wpool.tile([1, 128], F32, tag="ones1")
                nc.vector.memset(ones1[:], 1.0)
                bls = wpool.tile([1, 2 * A_DIM], F32, tag="bls")
                nc.scalar.dma_start(bls[:], bl_d[:])

            aSP = wpool.tile([128, G], F32, tag="aSP")
            aX0 = wpool.tile([128, G], F32, tag="aX0")
            aPD = wpool.tile([128, G], F32, tag="aPD")

            def stage3(t, h2h, h2l):
                for c in range(TB // 128):
                    g = t * (TB // 128) + c
                    cs = slice(c * 128, (c + 1) * 128)
                    # psum holds 2^22 * [x0 | x1]; one bank per matmul
                    px = psC.tile([128, 2 * A_DIM], F32, tag="px")
                    px0 = px[:, :A_DIM]
                    px1 = px[:, A_DIM:]
                    for half, ph in ((slice(0, A_DIM), px0),
                                     (slice(A_DIM, 2 * A_DIM), px1)):
                        nc.tensor.matmul(ph, h2h[:, cs], w3hs[:, half],
                                         start=True, stop=False)
                        nc.tensor.matmul(ph, h2h[:, cs], w3ls[:, half],
                                         start=False, stop=False)
                        nc.tensor.matmul(ph, h2l[:, cs], w3hs[:, half],
                                         start=False, stop=not with_bl)
                        if with_bl:
                            nc.tensor.matmul(ph, ones1[:], bls[:, half],
                                             start=False, stop=True)

                    x0s = cpool.tile([128, A_DIM], F32, tag="x0s")
                    nc.vector.tensor_scalar(out=x0s[:], in0=px0, scalar1=DS22,
                                            scalar2=0.0, op0=ALU.mult, op1=ALU.add,
                                            accum_out=aX0[:, g:g + 1])
                    d = cpool.tile([128, A_DIM], F32, tag="d")
                    nc.vector.scalar_tensor_tensor(
                        out=d[:], in0=px1, scalar=DS22,
                        in1=x0s[:], op0=ALU.mult, op1=ALU.subtract)
                    pred = cpool.tile([128, A_DIM], F32, tag="pred")
                    nc.vector.tensor_scalar(out=pred[:], in0=d[:], scalar1=0.0,
                                            scalar2=None, op0=ALU.is_gt)
                    nc.scalar.dma_start(pred_d[g * 128:(g + 1) * 128, :], pred[:])

                    proc = cpool.tile([128, NE], BF16, tag="proc")
                    nc.gpsimd.local_scatter(proc[:], ones20[:], idxs[:, g, :],
                                            channels=128, num_elems=NE,
                                            num_idxs=MAX_LEN)
                    pd = cpool.tile([128, A_DIM], F32, tag="pd")
                    nc.vector.scalar_tensor_tensor(
                        out=pd[:], in0=d[:], scalar=1.0, in1=proc[:, :A_DIM],
                        op0=ALU.mult, op1=ALU.mult, accum_out=aPD[:, g:g + 1])

                    # sum softplus(x0)+softplus(x1) in one accumulated pass
                    ex = cpool.tile([128, 2 * A_DIM], F32, tag="ex")
                    nc.scalar.activation(ex[:], px[:], AF.Exp, scale=DS22)
                    nc.scalar.activation(ex[:], ex[:], AF.Ln, bias=1.0,
                                         accum_out=aSP[:, g:g + 1])

            for t in range(nt):
                b0 = t * TB
                # ---- transposed, pre-split fp16 activations -------------
                shT = spool.tile([128, KT, TB], F16, tag="shT")
                slT = spool.tile([128, KT, TB], F16, tag="slT")
                for kt in range(KT):
                    nc.sync.dma_start(shT[:, kt, :],
                                      sh_d[kt * 128:(kt + 1) * 128, b0:b0 + TB])
                    nc.sync.dma_start(slT[:, kt, :],
                                      sl_d[kt * 128:(kt + 1) * 128, b0:b0 + TB])
                if t == 0:
                    load_late_weights()

                # ---- stage 1: h1 = relu(s @ W1 + b1), split hi/lo -------
                # m-groups sized to psA bufs with the kt loop outside: each
                # kt step needs only one 0.75MB slice of weights+activations,
                # so the first tile streams at HBM pace instead of stalling
                # on the full 6MB working set.
                h1h = hpool.tile([128, KT, TB], F16, tag="h1h")
                h1l = hpool.tile([128, KT, TB], F16, tag="h1l")
                h1f = hpool.tile([128, KT, TB], F32, tag="h1f")
                # tile 0 streams at HBM pace: kt-outer over m-pairs so each
                # kt step needs only a 0.75MB slice; later tiles are fully
                # prefetched and use the evac-hiding m-outer order.
                gsz = 2 if t == 0 else MT
                for mg in range(0, MT, gsz):
                    ms_group = list(range(mg, min(mg + gsz, MT)))
                    phs = {}
                    for m in ms_group:
                        ph1 = psA.tile([128, TB], F32, tag="ph1")
                        phs[m] = ph1
                    if t == 0:
                        for kt in range(KT):
                            for m in ms_group:
                                ms = slice(m * 128, (m + 1) * 128)
                                nc.tensor.matmul(phs[m][:], w1hs[:, kt, ms],
                                                 shT[:, kt, :], start=(kt == 0), stop=False)
                                nc.tensor.matmul(phs[m][:], w1hs[:, kt, ms],
                                                 slT[:, kt, :], start=False, stop=False)
                                nc.tensor.matmul(phs[m][:], w1ls[:, kt, ms],
                                                 shT[:, kt, :], start=False,
                                                 stop=(kt == KT - 1))
                    else:
                        for m in ms_group:
                            ms = slice(m * 128, (m + 1) * 128)
                            for kt in range(KT):
                                nc.tensor.matmul(phs[m][:], w1hs[:, kt, ms],
                                                 shT[:, kt, :], start=(kt == 0), stop=False)
                                nc.tensor.matmul(phs[m][:], w1hs[:, kt, ms],
                                                 slT[:, kt, :], start=False, stop=False)
                                nc.tensor.matmul(phs[m][:], w1ls[:, kt, ms],
                                                 shT[:, kt, :], start=False,
                                                 stop=(kt == KT - 1))
                    for m in ms_group:
                        ph1 = phs[m]
                        nc.scalar.activation(h1h[:, m, :], ph1[:], AF.Relu,
                                             bias=b1s[:, MT + m:MT + m + 1], scale=DS11)
                        nc.scalar.activation(h1f[:, m, :], ph1[:], AF.Relu,
                                             bias=b1s[:, m:m + 1], scale=DS22)
                        nc.vector.scalar_tensor_tensor(
                            out=h1l[:, m, :], in0=h1f[:, m, :], scalar=float(SC),
                            in1=h1h[:, m, :], op0=ALU.mult, op1=ALU.subtract)

                # ---- stage 2: h2 = prelu(h1 @ W2 + b2, 0.2), split ------
                ph2 = psB.tile([128, TB], F32, tag="ph2")
                for kt in range(KT):
                    nc.tensor.matmul(ph2[:], w2hs[:, kt, :], h1h[:, kt, :],
                                     start=(kt == 0), stop=False)
                    nc.tensor.matmul(ph2[:], w2ls[:, kt, :], h1h[:, kt, :],
                                     start=False, stop=False)
                    nc.tensor.matmul(ph2[:], w2hs[:, kt, :], h1l[:, kt, :],
                                     start=False, stop=(kt == KT - 1))
                h2h = h2pool.tile([128, TB], F16, tag="h2h")
                h2l = h2pool.tile([128, TB], F16, tag="h2l")
                h2f = h2pool.tile([128, TB], F32, tag="h2f")
                for c in range(TB // 128):
                    cs = slice(c * 128, (c + 1) * 128)
                    nc.scalar.activation(h2h[:, cs], ph2[:, cs], AF.Prelu,
                                         bias=b2s[:, 1:2], scale=DS11, alpha=0.2)
                    nc.scalar.activation(h2f[:, cs], ph2[:, cs], AF.Prelu,
                                         bias=b2s[:, 0:1], scale=DS22, alpha=0.2)
                    nc.vector.scalar_tensor_tensor(
                        out=h2l[:, cs], in0=h2f[:, cs], scalar=float(SC),
                        in1=h2h[:, cs], op0=ALU.mult, op1=ALU.subtract)

                stage3(t, h2h, h2l)

            # ---- loss accumulators: reduced on host ----------------------
            for t in range(nt):
                gs = slice(t * (TB // 128), (t + 1) * (TB // 128))
                nc.scalar.dma_start(acc_d[0, :, gs], aSP[:, gs])
                nc.scalar.dma_start(acc_d[1, :, gs], aX0[:, gs])
                nc.scalar.dma_start(acc_d[2, :, gs], aPD[:, gs])

    nc.compile()
    return nc


def _get(Bc, with_bl, with_b1):
    key = (Bc, with_bl, with_b1)
    if key not in _cache:
        _cache[key] = _build(Bc, with_bl, with_b1)
    return _cache[key]


def _split_w(w):
    f32 = np.float32
    wh = w.astype(np.float16)
    hi = (wh.astype(f32) * SC).astype(np.float16)
    lo = ((w - wh.astype(f32)) * SC).astype(np.float16)
    return hi, lo


def _prep(s, gold, pos, W1, b1, W2, b2, Wl, bl):
    f32 = np.float32
    s = np.ascontiguousarray(s, dtype=f32)
    sh = np.clip(s * SC, -65000.0, 65000.0).astype(np.float16)
    sl = ((s - sh.astype(f32) / SC) * SC).astype(np.float16)
    sh = np.ascontiguousarray(sh.T)      # device wants [S_DIM, B]
    sl = np.ascontiguousarray(sl.T)
    w1h, w1l = _split_w(np.ascontiguousarray(W1, f32))
    w2h, w2l = _split_w(np.ascontiguousarray(W2, f32))
    Wl = np.asarray(Wl, f32)
    w3 = np.concatenate([Wl[:, :, 0].T, Wl[:, :, 1].T], axis=1)  # [128, 1024]
    w3h, w3l = _split_w(np.ascontiguousarray(w3))

    gold = np.asarray(gold).astype(np.int64)
    pos = np.asarray(pos).astype(np.int64)
    L = gold.shape[1]
    mask = np.arange(L)[None, :] < pos[:, None]
    dup = ((gold[:, :, None] == gold[:, None, :])
           & np.tril(np.ones((L, L), bool), -1)[None]).any(axis=2)
    pad = A_DIM + np.arange(L)[None, :]
    idx = np.where(mask & ~dup, gold, pad).astype(np.int16)

    MT = H_DIM // 128
    b1 = np.asarray(b1, f32)
    b1_dev = np.concatenate([b1.reshape(MT, 128).T,
                             (b1 * SC).reshape(MT, 128).T], axis=1)
    b1_dev = np.ascontiguousarray(b1_dev)
    b2 = np.asarray(b2, f32)
    b2_dev = np.ascontiguousarray(np.stack([b2, b2 * SC], axis=1))
    bl = np.asarray(bl, f32)
    with_bl = bool(np.any(bl != 0))
    bl_dev = np.ascontiguousarray(
        np.concatenate([bl[:, 0], bl[:, 1]])[None, :] * (SC * SC))
    with_b1 = bool(np.any(b1 != 0))
    return (sh, sl, w1h, w1l, w2h, w2l, w3h, w3l, idx,
            b1_dev, b2_dev, with_bl, with_b1, bl_dev)


def _in_maps(inputs):
    s = np.asarray(inputs["s"])
    B = s.shape[0]
    assert B % N_CORES == 0
    Bc = B // N_CORES
    (sh, sl, w1h, w1l, w2h, w2l, w3h, w3l, idx,
     b1_dev, b2_dev, with_bl, with_b1, bl_dev) = _prep(
        s, inputs["a_target_gold"], inputs["s_target_pos"],
        inputs["W1"], inputs["b1"], inputs["W2"], inputs["b2"],
        inputs["Wl"], inputs["bl"])
    maps = []
    for c in range(N_CORES):
        r = slice(c * Bc, (c + 1) * Bc)
        m = {"sh": np.ascontiguousarray(sh[:, r]),
             "sl": np.ascontiguousarray(sl[:, r]),
             "w1h": w1h, "w1l": w1l, "w2h": w2h, "w2l": w2l,
             "w3h": w3h, "w3l": w3l,
             "b1": b1_dev, "b2": b2_dev, "idx": idx[r]}
        if with_bl:
            m["bl"] = bl_dev
        maps.append(m)
    return maps, Bc, (with_bl, with_b1), B


def _gather(res, B):
    pred = np.concatenate([res.results[c]["pred"] for c in range(N_CORES)], axis=0)
    total = 0.0
    for c in range(N_CORES):
        a = res.results[c]["acc"].astype(np.float64)
        total += a[0].sum() - a[1].sum() - a[2].sum()
    loss = np.float32(total / (B * 2 * A_DIM))
    return loss, pred


def kernel(s, a_target_gold, s_target_pos, beta, W1, b1, W2, b2, Wl, bl):
    inputs = dict(s=s, a_target_gold=a_target_gold, s_target_pos=s_target_pos,
                  W1=W1, b1=b1, W2=W2, b2=b2, Wl=Wl, bl=bl)
    maps, Bc, flags, B = _in_maps(inputs)
    nc = _get(Bc, *flags)
    res = run_bass_kernel_spmd(nc, maps, core_ids=list(range(N_CORES)))
    loss, pred = _gather(res, B)
    return (loss, pred)


def run_traced(**inputs):
    """kernel() but with NTFF tracing; returns (outputs, BassKernelResults)."""
    maps, Bc, flags, B = _in_maps(inputs)
    nc = _get(Bc, *flags)
    res = run_bass_kernel_spmd(nc, maps, core_ids=list(range(N_CORES)), trace=True)
    loss, pred = _gather(res, B)
    return (loss, pred), res


# revision 33
# speedup vs baseline: 1.0113x; 1.0113x over previous
"""Trainium2 Bass kernel for nn_DiaMultiDense.

Computes, for s:[B,1024] f32, gold:[B,20] int, pos:[B] int and MLP weights:
    h  = relu(s @ W1 + b1)
    h  = leaky_relu(h @ W2 + b2, 0.2)
    logits[b,a,w] = h @ Wl[a,:,w] + bl[a,w]          (A=512, w in {0,1})
    pred[b,a] = argmax_w logits[b,a,w]               (-> float 0/1)
    proc[b,a] = 1 if a in gold[b, :pos[b]] else 0
    tgt = one_hot pairs of proc; loss = -mean(tgt*logsig(x) + (1-tgt)*logsig(-x))
Returns (loss, pred).

Strategy: pure data parallel over 8 NeuronCores (2048 rows each).  All
three matmul stages run as 3-pass fp16 hi/lo decompositions on the PE
(1 cyc/row vs 4 for fp32) with power-of-two scaling (x2048 per operand,
products at 2^22) that keeps every fp16 operand in normal range; the
hi/lo splits of the intermediate activations are produced on ACT/DVE
off the critical path.  This reproduces fp32-level logits (~1 argmax
flip in 8.4M vs an exact-fp32 oracle).  Loss terms are accumulated
on-chip (softplus via Exp+Ln LUTs, row-sums via ACT accumulators); the
per-row target union mask is built with a gpsimd local_scatter of ones
at host-deduplicated indices, with masked/duplicate slots redirected to
pad columns 512..531.  Host work is only sharding, dtype/layout
marshalling of inputs, and the final 8-way scalar reduction.
"""

import sys

sys.path.insert(0, "/opt/trn_rl_repo")

import numpy as np

import concourse.bacc as bacc
import concourse.mybir as mybir
import concourse.tile as tile
from concourse.bass_utils import run_bass_kernel_spmd

AF = mybir.ActivationFunctionType

# Pin every ACT function we use to the one LUT set that contains them all
# (natural_log_exp_and_others).  The default first-match table assignment
# alternates between act-func sets (Ln -> natural_log, Exp -> ..._exp_...),
# inserting ~40 mid-kernel ACT_TABLE_LOADs (~51us).  Removing our funcs
# from every other set makes the keep-set the unique match; set contents
# are only used for placement, the runtime table bytes come from
# act_info.json by index, so this is purely a scheduling hint.
_KEEP_TABLE = "natural_log_exp_and_others"
_PINNED = {AF.Relu, AF.Prelu, AF.Identity, AF.Exp, AF.Ln, AF.Copy}


def _pinned_tables(arch):
    from concourse.hw_specs import get_activation_tables
    orig = get_activation_tables(arch)
    return {name: (set(funcs) if name == _KEEP_TABLE else set(funcs) - _PINNED)
            for name, funcs in orig.items()}


bacc.get_activation_tables = _pinned_tables

ALU = mybir.AluOpType
F32 = mybir.dt.float32
F16 = mybir.dt.float16
BF16 = mybir.dt.bfloat16
I16 = mybir.dt.int16

B_FULL = 16384
S_DIM = 1024
H_DIM = 1024
H4 = 128
A_DIM = 512
MAX_LEN = 20
N_CORES = 8
TB = 512                      # batch rows per pipeline tile
NE = 544                      # scatter row width: 512 actions + 32 pad slots
SC = np.float32(2048.0)       # 2**11 operand scaling for the fp16 splits
DS11 = float(2.0 ** -11)      # psum(2^22) -> hi-split(2^11) descale
DS22 = float(2.0 ** -22)      # psum(2^22) -> true value descale

_cache = {}


def _build(Bc, with_bl, with_b1):
    nt = Bc // TB
    KT = S_DIM // 128          # contraction tiles for stages 1/2
    G = Bc // 128              # 128-row output chunks per core
    MT = H_DIM // 128

    nc = bacc.Bacc(None, target_bir_lowering=False)

    sh_d = nc.dram_tensor("sh", [S_DIM, Bc], F16, kind="ExternalInput")
    sl_d = nc.dram_tensor("sl", [S_DIM, Bc], F16, kind="ExternalInput")
    w1h_d = nc.dram_tensor("w1h", [S_DIM, H_DIM], F16, kind="ExternalInput")
    w1l_d = nc.dram_tensor("w1l", [S_DIM, H_DIM], F16, kind="ExternalInput")
    w2h_d = nc.dram_tensor("w2h", [H_DIM, H4], F16, kind="ExternalInput")
    w2l_d = nc.dram_tensor("w2l", [H_DIM, H4], F16, kind="ExternalInput")
    # [We_hi, Wo_hi] and [We_lo, Wo_lo] stacked on the free dim
    w3h_d = nc.dram_tensor("w3h", [H4, 2 * A_DIM], F16, kind="ExternalInput")
    w3l_d = nc.dram_tensor("w3l", [H4, 2 * A_DIM], F16, kind="ExternalInput")
    b1_d = nc.dram_tensor("b1", [128, 2 * MT], F32, kind="ExternalInput")  # [b1, b1*2048]
    b2_d = nc.dram_tensor("b2", [128, 2], F32, kind="ExternalInput")       # [b2, b2*2048]
    idx_d = nc.dram_tensor("idx", [Bc, MAX_LEN], I16, kind="ExternalInput")
    if with_bl:
        bl_d = nc.dram_tensor("bl", [1, 2 * A_DIM], F32, kind="ExternalInput")  # *2^22
    pred_d = nc.dram_tensor("pred", [Bc, A_DIM], F32, kind="ExternalOutput")
    acc_d = nc.dram_tensor("acc", [3, 128, G], F32, kind="ExternalOutput")

    with tile.TileContext(nc) as tc:
        with (
            tc.tile_pool(name="wpool", bufs=1) as wpool,
            tc.tile_pool(name="spool", bufs=3) as spool,
            tc.tile_pool(name="hpool", bufs=1) as hpool,
            tc.tile_pool(name="h2pool", bufs=1) as h2pool,
            tc.tile_pool(name="cpool", bufs=3) as cpool,
            tc.tile_pool(name="psA", bufs=3, space="PSUM") as psA,
            tc.tile_pool(name="psB", bufs=1, space="PSUM") as psB,
            tc.tile_pool(name="psC", bufs=2, space="PSUM") as psC,
        ):
            # ---- persistent weights / constants -------------------------
            # stage-1 weights on two queues so the sync queue only carries
            # the streamed activations
            w1hs = wpool.tile([128, KT, H_DIM], F16, tag="w1hs")
            w1ls = wpool.tile([128, KT, H_DIM], F16, tag="w1ls")
            for kt in range(KT):
                nc.scalar.dma_start(w1hs[:, kt, :], w1h_d[kt * 128:(kt + 1) * 128, :])
                nc.gpsimd.dma_start(w1ls[:, kt, :], w1l_d[kt * 128:(kt + 1) * 128, :])
            b1s = wpool.tile([128, 2 * MT], F32, tag="b1s")
            nc.scalar.dma_start(b1s[:], b1_d[:])
            b2s = wpool.tile([128, 2], F32, tag="b2s")
            nc.scalar.dma_start(b2s[:], b2_d[:])
            w2hs = wpool.tile([128, KT, H4], F16, tag="w2hs")
            w2ls = wpool.tile([128, KT, H4], F16, tag="w2ls")
            w3hs = wpool.tile([128, 2 * A_DIM], F16, tag="w3hs")
            w3ls = wpool.tile([128, 2 * A_DIM], F16, tag="w3ls")
            idxs = wpool.tile([128, G, MAX_LEN], I16, tag="idxs")

            def load_late_weights():
                # stage-2/3 weights + indices (~1.2MB) are first used ~60us
                # in; deferring their DMA emission keeps the HBM-bound tile-0
                # ramp dedicated to stage-1 weights and activations.
                nc.scalar.dma_start(w2hs[:], w2h_d.rearrange("(t p) m -> p t m", p=128))
                nc.scalar.dma_start(w2ls[:], w2l_d.rearrange("(t p) m -> p t m", p=128))
                nc.scalar.dma_start(w3hs[:], w3h_d[:])
                nc.scalar.dma_start(w3ls[:], w3l_d[:])
                nc.scalar.dma_start(idxs[:], idx_d.rearrange("(g p) l -> p g l", p=128))
            ones20 = wpool.tile([128, MAX_LEN], BF16, tag="ones20")
            nc.vector.memset(ones20[:], 1.0)
            if with_bl:
                ones1 = wpool.tile([1, 128], F32, tag="ones1")
                nc.vector.memset(ones1[:], 1.0)
                bls = wpool.tile([1, 2 * A_DIM], F32, tag="bls")
                nc.scalar.dma_start(bls[:], bl_d[:])

            aSP = wpool.tile([128, G], F32, tag="aSP")
            aX0 = wpool.tile([128, G], F32, tag="aX0")
            aPD = wpool.tile([128, G], F32, tag="aPD")

            def stage3(t, h2h, h2l):
                for c in range(TB // 128):
                    g = t * (TB // 128) + c
                    cs = slice(c * 128, (c + 1) * 128)
                    # psum holds 2^22 * [x0 | d]; pred/loss-dot need only d
                    # at full precision, x0 feeds only the loss -> single hi
                    # pass (2^-11 error, ~1e-5 rel on the loss): 4 MMs not 6
                    px = psC.tile([128, 2 * A_DIM], F32, tag="px")
                    px0 = px[:, :A_DIM]
                    px1 = px[:, A_DIM:]
                    h0 = slice(0, A_DIM)
                    h1_ = slice(A_DIM, 2 * A_DIM)
                    nc.tensor.matmul(px0, h2h[:, cs], w3hs[:, h0],
                                     start=True, stop=not with_bl)
                    nc.tensor.matmul(px1, h2h[:, cs], w3hs[:, h1_],
                                     start=True, stop=False)
                    nc.tensor.matmul(px1, h2h[:, cs], w3ls[:, h1_],
                                     start=False, stop=False)
                    nc.tensor.matmul(px1, h2l[:, cs], w3hs[:, h1_],
                                     start=False, stop=not with_bl)
                    if with_bl:
                        nc.tensor.matmul(px0, ones1[:], bls[:, h0],
                                         start=False, stop=True)
                        nc.tensor.matmul(px1, ones1[:], bls[:, h1_],
                                         start=False, stop=True)

                    x0s = cpool.tile([128, A_DIM], F32, tag="x0s")
                    nc.vector.tensor_scalar(out=x0s[:], in0=px0, scalar1=DS22,
                                            scalar2=0.0, op0=ALU.mult, op1=ALU.add,
                                            accum_out=aX0[:, g:g + 1])
                    pred = cpool.tile([128, A_DIM], F32, tag="pred")
                    nc.vector.tensor_scalar(out=pred[:], in0=px1, scalar1=0.0,
                                            scalar2=None, op0=ALU.is_gt)
                    nc.scalar.dma_start(pred_d[g * 128:(g + 1) * 128, :], pred[:])

                    proc = cpool.tile([128, NE], BF16, tag="proc")
                    nc.gpsimd.local_scatter(proc[:], ones20[:], idxs[:, g, :],
                                            channels=128, num_elems=NE,
                                            num_idxs=MAX_LEN)
                    pd = cpool.tile([128, A_DIM], F32, tag="pd")
                    nc.vector.scalar_tensor_tensor(
                        out=pd[:], in0=px1, scalar=DS22, in1=proc[:, :A_DIM],
                        op0=ALU.mult, op1=ALU.mult, accum_out=aPD[:, g:g + 1])

                    # x1 = x0 + d only for softplus; one Ln pass over both
                    x1s = cpool.tile([128, A_DIM], F32, tag="x1s")
                    nc.vector.scalar_tensor_tensor(
                        out=x1s[:], in0=px1, scalar=DS22,
                        in1=x0s[:], op0=ALU.mult, op1=ALU.add)
                    ex = cpool.tile([128, 2 * A_DIM], F32, tag="ex")
                    nc.scalar.activation(ex[:, h0], px0, AF.Exp, scale=DS22)
                    nc.scalar.activation(ex[:, h1_], x1s[:], AF.Exp)
                    nc.scalar.activation(ex[:], ex[:], AF.Ln, bias=1.0,
                                         accum_out=aSP[:, g:g + 1])

            for t in range(nt):
                b0 = t * TB
                # ---- transposed, pre-split fp16 activations -------------
                shT = spool.tile([128, KT, TB], F16, tag="shT")
                slT = spool.tile([128, KT, TB], F16, tag="slT")
                for kt in range(KT):
                    nc.sync.dma_start(shT[:, kt, :],
                                      sh_d[kt * 128:(kt + 1) * 128, b0:b0 + TB])
                    nc.sync.dma_start(slT[:, kt, :],
                                      sl_d[kt * 128:(kt + 1) * 128, b0:b0 + TB])
                if t == 0:
                    load_late_weights()

                # ---- stage 1: h1 = relu(s @ W1 + b1), split hi/lo -------
                # m-groups sized to psA bufs with the kt loop outside: each
                # kt step needs only one 0.75MB slice of weights+activations,
                # so the first tile streams at HBM pace instead of stalling
                # on the full 6MB working set.
                h1h = hpool.tile([128, KT, TB], F16, tag="h1h")
                h1l = hpool.tile([128, KT, TB], F16, tag="h1l")
                h1f = hpool.tile([128, KT, TB], F32, tag="h1f")
                # tile 0 streams at HBM pace: kt-outer over m-pairs so each
                # kt step needs only a 0.75MB slice; later tiles are fully
                # prefetched and use the evac-hiding m-outer order.
                gsz = 2 if t == 0 else MT
                for mg in range(0, MT, gsz):
                    ms_group = list(range(mg, min(mg + gsz, MT)))
                    phs = {}
                    for m in ms_group:
                        ph1 = psA.tile([128, TB], F32, tag="ph1")
                        phs[m] = ph1
                    if t == 0:
                        for kt in range(KT):
                            for m in ms_group:
                                ms = slice(m * 128, (m + 1) * 128)
                                nc.tensor.matmul(phs[m][:], w1hs[:, kt, ms],
                                                 shT[:, kt, :], start=(kt == 0), stop=False)
                                nc.tensor.matmul(phs[m][:], w1hs[:, kt, ms],
                                                 slT[:, kt, :], start=False, stop=False)
                                nc.tensor.matmul(phs[m][:], w1ls[:, kt, ms],
                                                 shT[:, kt, :], start=False,
                                                 stop=(kt == KT - 1))
                    else:
                        for m in ms_group:
                            ms = slice(m * 128, (m + 1) * 128)
                            for kt in range(KT):
                                nc.tensor.matmul(phs[m][:], w1hs[:, kt, ms],
                                                 shT[:, kt, :], start=(kt == 0), stop=False)
                                nc.tensor.matmul(phs[m][:], w1hs[:, kt, ms],
                                                 slT[:, kt, :], start=False, stop=False)
                                nc.tensor.matmul(phs[m][:], w1ls[:, kt, ms],
                                                 shT[:, kt, :], start=False,
                                                 stop=(kt == KT - 1))
                    for m in ms_group:
                        ph1 = phs[m]
                        nc.scalar.activation(h1h[:, m, :], ph1[:], AF.Relu,
                                             bias=b1s[:, MT + m:MT + m + 1], scale=DS11)
                        nc.scalar.activation(h1f[:, m, :], ph1[:], AF.Relu,
                                             bias=b1s[:, m:m + 1], scale=DS22)
                        nc.vector.scalar_tensor_tensor(
                            out=h1l[:, m, :], in0=h1f[:, m, :], scalar=float(SC),
                            in1=h1h[:, m, :], op0=ALU.mult, op1=ALU.subtract)

                # ---- stage 2: h2 = prelu(h1 @ W2 + b2, 0.2), split ------
                ph2 = psB.tile([128, TB], F32, tag="ph2")
                for kt in range(KT):
                    nc.tensor.matmul(ph2[:], w2hs[:, kt, :], h1h[:, kt, :],
                                     start=(kt == 0), stop=False)
                    nc.tensor.matmul(ph2[:], w2ls[:, kt, :], h1h[:, kt, :],
                                     start=False, stop=False)
                    nc.tensor.matmul(ph2[:], w2hs[:, kt, :], h1l[:, kt, :],
                                     start=False, stop=(kt == KT - 1))
                h2h = h2pool.tile([128, TB], F16, tag="h2h")
                h2l = h2pool.tile([128, TB], F16, tag="h2l")
                h2f = h2pool.tile([128, TB], F32, tag="h2f")
                for c in range(TB // 128):
                    cs = slice(c * 128, (c + 1) * 128)
                    nc.scalar.activation(h2h[:, cs], ph2[:, cs], AF.Prelu,
                                         bias=b2s[:, 1:2], scale=DS11, alpha=0.2)
                    nc.scalar.activation(h2f[:, cs], ph2[:, cs], AF.Prelu,
                                         bias=b2s[:, 0:1], scale=DS22, alpha=0.2)
                    nc.vector.scalar_tensor_tensor(
                        out=h2l[:, cs], in0=h2f[:, cs], scalar=float(SC),
                        in1=h2h[:, cs], op0=ALU.mult, op1=ALU.subtract)

                stage3(t, h2h, h2l)

            # ---- loss accumulators: reduced on host ----------------------
            for t in range(nt):
                gs = slice(t * (TB // 128), (t + 1) * (TB // 128))
                nc.scalar.dma_start(acc_d[0, :, gs], aSP[:, gs])
                nc.scalar.dma_start(acc_d[1, :, gs], aX0[:, gs])
                nc.scalar.dma_start(acc_d[2, :, gs], aPD[:, gs])

    nc.compile()
    return nc


def _get(Bc, with_bl, with_b1):
    key = (Bc, with_bl, with_b1)
    if key not in _cache:
        _cache[key] = _build(Bc, with_bl, with_b1)
    return _cache[key]


def _split_w(w):
    f32 = np.float32
    wh = w.astype(np.float16)
    hi = (wh.astype(f32) * SC).astype(np.float16)
    lo = ((w - wh.astype(f32)) * SC).astype(np.float16)
    return hi, lo


def _prep(s, gold, pos, W1, b1, W2, b2, Wl, bl):
    f32 = np.float32
    s = np.ascontiguousarray(s, dtype=f32)
    sh = np.clip(s * SC, -65000.0, 65000.0).astype(np.float16)
    sl = ((s - sh.astype(f32) / SC) * SC).astype(np.float16)
    sh = np.ascontiguousarray(sh.T)      # device wants [S_DIM, B]
    sl = np.ascontiguousarray(sl.T)
    w1h, w1l = _split_w(np.ascontiguousarray(W1, f32))
    w2h, w2l = _split_w(np.ascontiguousarray(W2, f32))
    Wl = np.asarray(Wl, f32)
    w3 = np.concatenate([Wl[:, :, 0].T, Wl[:, :, 1].T - Wl[:, :, 0].T], axis=1)
    w3h, w3l = _split_w(np.ascontiguousarray(w3))   # [We | Wo-We] hi/lo

    gold = np.asarray(gold).astype(np.int64)
    pos = np.asarray(pos).astype(np.int64)
    L = gold.shape[1]
    mask = np.arange(L)[None, :] < pos[:, None]
    dup = ((gold[:, :, None] == gold[:, None, :])
           & np.tril(np.ones((L, L), bool), -1)[None]).any(axis=2)
    pad = A_DIM + np.arange(L)[None, :]
    idx = np.where(mask & ~dup, gold, pad).astype(np.int16)

    MT = H_DIM // 128
    b1 = np.asarray(b1, f32)
    b1_dev = np.concatenate([b1.reshape(MT, 128).T,
                             (b1 * SC).reshape(MT, 128).T], axis=1)
    b1_dev = np.ascontiguousarray(b1_dev)
    b2 = np.asarray(b2, f32)
    b2_dev = np.ascontiguousarray(np.stack([b2, b2 * SC], axis=1))
    bl = np.asarray(bl, f32)
    with_bl = bool(np.any(bl != 0))
    bl_dev = np.ascontiguousarray(
        np.concatenate([bl[:, 0], bl[:, 1] - bl[:, 0]])[None, :] * (SC * SC))
    with_b1 = bool(np.any(b1 != 0))
    return (sh, sl, w1h, w1l, w2h, w2l, w3h, w3l, idx,
            b1_dev, b2_dev, with_bl, with_b1, bl_dev)


def _in_maps(inputs):
    s = np.asarray(inputs["s"])
    B = s.shape[0]
    assert B % N_CORES == 0
    Bc = B // N_CORES
    (sh, sl, w1h, w1l, w2h, w2l, w3h, w3l, idx,
     b1_dev, b2_dev, with_bl, with_b1, bl_dev) = _prep(
        s, inputs["a_target_gold"], inputs["s_target_pos"],
        inputs["W1"], inputs["b1"], inputs["W2"], inputs["b2"],
        inputs["Wl"], inputs["bl"])
    maps = []
    for c in range(N_CORES):
        r = slice(c * Bc, (c + 1) * Bc)
        m = {"sh": np.ascontiguousarray(sh[:, r]),
             "sl": np.ascontiguousarray(sl[:, r]),
             "w1h": w1h, "w1l": w1l, "w2h": w2h, "w2l": w2l,
             "w3h": w3h, "w3l": w3l,
             "b1": b1_dev, "b2": b2_dev, "idx": idx[r]}
        if with_bl:
            m["bl"] = bl_dev
        maps.append(m)
    return maps, Bc, (with_bl, with_b1), B


def _gather(res, B):
    pred = np.concatenate([res.results[c]["pred"] for c in range(N_CORES)], axis=0)
    total = 0.0
    for c in range(N_CORES):
        a = res.results[c]["acc"].astype(np.float64)
        total += a[0].sum() - a[1].sum() - a[2].sum()
    loss = np.float32(total / (B * 2 * A_DIM))
    return loss, pred


def kernel(s, a_target_gold, s_target_pos, beta, W1, b1, W2, b2, Wl, bl):
    inputs = dict(s=s, a_target_gold=a_target_gold, s_target_pos=s_target_pos,
                  W1=W1, b1=b1, W2=W2, b2=b2, Wl=Wl, bl=bl)
    maps, Bc, flags, B = _in_maps(inputs)
    nc = _get(Bc, *flags)
    res = run_bass_kernel_spmd(nc, maps, core_ids=list(range(N_CORES)))
    loss, pred = _gather(res, B)
    return (loss, pred)


def run_traced(**inputs):
    """kernel() but with NTFF tracing; returns (outputs, BassKernelResults)."""
    maps, Bc, flags, B = _in_maps(inputs)
    nc = _get(Bc, *flags)
    res = run_bass_kernel_spmd(nc, maps, core_ids=list(range(N_CORES)), trace=True)
    loss, pred = _gather(res, B)
    return (loss, pred), res
